# revision 11
# baseline (speedup 1.0000x reference)
"""KANLinear Trainium2 kernel — Derivative_Erf-feature + fp8 DoubleRow version.

Strategy:
  - Spline branch: the 8 cardinal cubic B-spline basis functions B(y-j)
    (uniform knots) are approximated by 8 Gaussians exp(-k(y-mu_c)^2),
    mu_c = 2..9, k=1.3, fitted by density-weighted least squares on the
    host (rms residual ~2e-3 of basis scale).  The 8x8 recombination A is
    folded into the spline weights, so the spline branch is a dense
    matmul over K = 8*1024 Gaussian features.
  - Each Gaussian feature is computed in ONE ACT op via Derivative_Erf:
    DErf(s*x + b) = (2/sqrt(pi)) * exp(-(s*x+b)^2), written as fp8
    directly.  No DVE subtract/square chain at all.
  - Base branch: Silu ACT table directly (exact silu), fp16 matmul.
  - The spline matmul runs in fp8 (e4m3) DoubleRow (2 k-groups/pass) at
    FD=512.  Weights scaled by S_W to sit in fp8 range; descale in the
    psum drain.
  - ACT table sets: Derivative_Erf and Silu live in different sets; a
    tiny DVE-produced bias tile makes the 8 silu ops depend on the last
    derf op, so ACT order is [64 derf][8 silu] per rep = 2 table loads.
  - Data-parallel over batch: 8 cores x 1024 rows.
"""
import numpy as np
import ml_dtypes

P = 128
NCORES = 8
BATCH, IN_F, OUT_F = 8192, 1024, 1024
B_LOC = BATCH // NCORES          # 1024
N_IC = IN_F // P                 # 8 input-feature chunks
N_OC = OUT_F // P                # 8 output chunks
NG = 8                           # gaussian centers
NCP = NG // 2                    # DoubleRow center pairs
K_G = 1.3                        # gaussian width (y units)

# grid constants (uniform knots; matches reference setup)
GRID_SIZE, SPLINE_ORDER = 5, 3
GRID_LO, GRID_HI = -1.0, 1.0
H = (GRID_HI - GRID_LO) / GRID_SIZE                      # 0.4
T0 = GRID_LO - SPLINE_ORDER * H                          # -2.2
MU_Y = np.arange(2.0, 2.0 + NG)                          # y-space centers
X_MU = (T0 + H * MU_Y).astype(np.float64)                # x-space centers
KP = K_G / (H * H)                                       # x-space width
SC = float(np.sqrt(KP))                                  # derf input scale

_BUILT = {}
_SW = 2048.0   # weight scale; host-verified to keep |w*S_W| < 240


def _fit_A():
    """Fit 8 derf-gaussians to the 8 cardinal basis fns, density weighted."""
    y = np.linspace(-2.0, 13.0, 6001)
    w = np.exp(-0.5 * (H * y + T0) ** 2)        # x-density at y
    sw = np.sqrt(w)
    t = y[:, None] - np.arange(8)[None, :]
    v = 2.0 - np.abs(t - 2.0)
    r1 = np.maximum(v, 0.0); r2 = np.maximum(v - 1.0, 0.0)
    T = (r1 ** 3 - 4.0 * r2 ** 3) / 6.0          # [N, 8] targets
    G = (2.0 / np.sqrt(np.pi)) * np.exp(
        -K_G * (y[:, None] - MU_Y[None, :]) ** 2)          # [N, 8] derf feats
    A, *_ = np.linalg.lstsq(G * sw[:, None], T * sw[:, None], rcond=None)
    return A                                      # [centers, basis]


def _build_nc(repeat=1, unroll=8):
    import concourse.bacc as bacc
    import concourse.mybir as mybir
    from concourse import tile

    AF = mybir.ActivationFunctionType
    ALU = mybir.AluOpType
    F32 = mybir.dt.float32
    F16 = mybir.dt.float16
    FP8 = mybir.dt.float8e4

    descale = float(1.0 / _SW)

    while repeat % unroll:
        unroll //= 2
    unroll = max(unroll, 1)

    nc = bacc.Bacc("TRN2", target_bir_lowering=False, debug=False)

    x_d = nc.dram_tensor("x16", [N_IC, P, B_LOC], F16, kind="ExternalInput")
    wb_d = nc.dram_tensor("wb", [N_OC, P, N_IC, P], F16, kind="ExternalInput")
    wg_d = nc.dram_tensor("wg", [N_OC, P, N_IC * NCP, 2, P], FP8,
                          kind="ExternalInput")
    out_d = nc.dram_tensor("out", [N_OC, P, B_LOC], F32, kind="ExternalOutput")

    with tile.TileContext(nc) as tc:
        with (
            tc.tile_pool(name="consts", bufs=1) as cpool,
            tc.tile_pool(name="xr", bufs=10) as xr,
            tc.tile_pool(name="silp", bufs=2) as silp,
            tc.tile_pool(name="zbp", bufs=2) as zbp,
            tc.tile_pool(name="gp", bufs=2) as gp,
            tc.tile_pool(name="wbp", bufs=1) as wbp,
            tc.tile_pool(name="wgp", bufs=2) as wgp,
            tc.tile_pool(name="op", bufs=2) as op,
            tc.tile_pool(name="psum", bufs=4, space="PSUM") as pp,
        ):
            # per-center derf bias consts
            bc = []
            for c in range(NG):
                b = cpool.tile([P, 1], F32, name=f"bc{c}")
                nc.any.memset(b[:], float(-SC * X_MU[c]))
                bc.append(b)

            # define psum buffers before the loop so iteration-0 "drains of
            # the previous rep" read initialized memory
            for i in range(4):
                t = pp.tile([P, 1024], F32, name=f"pginit{i}", tag="pg")
                nc.vector.memset(t[:], 0.0)

            def emit_drain(prev_pg, oc, rep):
                ot = op.tile([P, 1024], F32, name=f"o{oc}_{rep}", tag="o")
                nc.vector.tensor_scalar(ot[:], prev_pg[oc][:], descale,
                                        None, ALU.mult)
                nc.sync.dma_start(out_d[oc], ot[:])

            def emit_feat(rep):
                """Feature block: x DMA + derf gaussians + silu."""
                xts, gt = [], []
                for ic in range(N_IC):
                    xt = xr.tile([P, B_LOC], F16, name=f"x{ic}_{rep}", tag="x")
                    nc.gpsimd.dma_start(xt[:], x_d[ic])
                    xts.append(xt)
                for ic in range(N_IC):
                    xt = xts[ic]
                    g = gp.tile([P, NG, B_LOC], FP8, name=f"g{ic}_{rep}",
                                tag=f"g{ic}")
                    for c in range(NG):
                        nc.scalar.activation(g[:, c, :], xt[:],
                                             AF.Derivative_Erf,
                                             bias=bc[c][:], scale=SC)
                    gt.append(g)

                # force [derf x64][silu x8] ACT order: silu bias depends on
                # the last derf output (value is exactly 0)
                zb = zbp.tile([P, 1], F32, name=f"zb_{rep}", tag="zb")
                nc.vector.tensor_scalar(zb[:], gt[-1][:, NG - 1, 0:1], 0.0,
                                        None, ALU.mult)
                sil = []
                for ic in range(N_IC):
                    st = silp.tile([P, B_LOC], F16, name=f"sil{ic}_{rep}",
                                   tag=f"s{ic}")
                    nc.scalar.activation(st[:], xts[ic][:], AF.Silu,
                                         bias=zb[:], scale=1.0)
                    sil.append(st)
                return gt, sil

            def emit_mms(rep, feat, prev_pg):
                """Matmul block consuming a feature set; prev_pg: psum tiles
                of the previous rep (oc4..7 still undrained) or None."""
                gt, sil = feat
                if prev_pg is not None:
                    for oc in range(4, 8):
                        emit_drain(prev_pg, oc, rep)

                def mm_oc(oc):
                    wb = wbp.tile([P, N_IC, P], F16, name=f"wb{oc}_{rep}",
                                  tag="wb")
                    nc.sync.dma_start(wb[:], wb_d[oc])
                    wg = wgp.tile([P, N_IC * NCP, 2, P], FP8,
                                  name=f"wg{oc}_{rep}", tag="wg")
                    nc.sync.dma_start(wg[:], wg_d[oc])
                    pg = pp.tile([P, 1024], F32, name=f"pg{oc}_{rep}",
                                 tag="pg")
                    # spline MMs first: they only need the derf block
                    for icp in range(N_IC * NCP):
                        ic, cp = divmod(icp, NCP)
                        for q in range(2):
                            nc.tensor.matmul(
                                pg[:, q * 512:(q + 1) * 512],
                                wg[:, icp, :, :],
                                gt[ic][:, 2 * cp:2 * cp + 2,
                                       q * 512:(q + 1) * 512],
                                start=(icp == 0), stop=False,
                                perf_mode=mybir.MatmulPerfMode.DoubleRow,
                                skip_group_check=True)
                    for ic in range(N_IC):
                        for bh in range(2):
                            nc.tensor.matmul(
                                pg[:, bh * 512:(bh + 1) * 512], wb[:, ic, :],
                                sil[ic][:, bh * 512:(bh + 1) * 512],
                                start=False, stop=(ic == N_IC - 1),
                                skip_group_check=True)
                    return pg

                pgs = {}
                for oc in range(4):
                    pgs[oc] = mm_oc(oc)
                for oc in range(4):
                    emit_drain(pgs, oc, rep)
                for oc in range(4, 8):
                    pgs[oc] = mm_oc(oc)
                return pgs

            if repeat == 1:
                f0 = emit_feat(0)
                pgs = emit_mms(0, f0, None)
                for oc in range(4, 8):
                    emit_drain(pgs, oc, "tail")
            else:
                # software pipeline: the feature block for rep r runs while
                # the matmuls consume rep r-1's features.  The pre-loop
                # feature block occupies the same rotation slots as the last
                # in-window block (unroll even), so the loop edge lines up.
                feat_pre = emit_feat("pre")
                with tc.For_i(0, repeat // unroll, 1):
                    prev_f = feat_pre
                    prev = None
                    for w in range(unroll):
                        f = emit_feat(w)
                        prev = emit_mms(w, prev_f, prev)
                        prev_f = f
                    for oc in range(4, 8):
                        emit_drain(prev, oc, "tail")

    nc.compile()
    return nc


def _prep(x, grid, base_weight, spline_weight, spline_scaler):
    # x transposed + fp16
    xT = np.ascontiguousarray(np.asarray(x, np.float32).T).astype(np.float16)

    # base weights: lhsT [oc][ic][p(k), m(out)], pre-scaled by S_W to share
    # the spline psum accumulation
    bw = np.asarray(base_weight, np.float64)             # [out, in]
    wb = (_SW * bw.T).reshape(N_IC, P, N_OC, P)
    wb = np.ascontiguousarray(wb.transpose(2, 1, 0, 3)).astype(np.float16)

    # gaussian weights: W~[i, c, o] = sum_j A[c, j] * (spline_w * scaler)
    A = _fit_A()                                         # [c, j]
    swsc = (np.asarray(spline_weight, np.float64)
            * np.asarray(spline_scaler, np.float64)[:, None, :])  # [in, 8, out]
    Wg = np.einsum('cj,ijo->ico', A, swsc)               # [in, c, out]
    Wgs = Wg * _SW                   # |Wgs| maxes well inside e4m3 range (240)
    # layout [oc, ic*NCP + cp, p, g, m]
    Wgs = Wgs.reshape(N_IC, P, NCP, 2, N_OC, P)          # [ic, p, cp, g, oc, m]
    wg = np.ascontiguousarray(Wgs.transpose(4, 1, 0, 2, 3, 5)).reshape(
        N_OC, P, N_IC * NCP, 2, P).astype(ml_dtypes.float8_e4m3)

    return xT, wb, wg


def _run(nc, in_maps):
    from concourse.bass_utils import run_bass_kernel_spmd
    return run_bass_kernel_spmd(nc, in_maps, core_ids=list(range(NCORES)))


def kernel(x, grid, base_weight, spline_weight, spline_scaler, _repeat=1):
    xT, wb, wg = _prep(x, grid, base_weight, spline_weight, spline_scaler)

    if _repeat not in _BUILT:
        _BUILT[_repeat] = _build_nc(_repeat)
    nc = _BUILT[_repeat]

    in_maps = []
    for c in range(NCORES):
        xs = np.ascontiguousarray(
            xT[:, c * B_LOC:(c + 1) * B_LOC].reshape(N_IC, P, B_LOC))
        in_maps.append({"x16": xs, "wb": wb, "wg": wg})

    res = _run(nc, in_maps)

    out = np.empty((BATCH, OUT_F), np.float32)
    for c in range(NCORES):
        o = res.results[c]["out"].reshape(OUT_F, B_LOC)   # [out, b_loc]
        out[c * B_LOC:(c + 1) * B_LOC, :] = o.T
    return out


# revision 12
# speedup vs baseline: 1.0942x; 1.0942x over previous
"""KANLinear Trainium2 kernel — Derivative_Erf-feature + fp8 DoubleRow version.

Strategy:
  - Spline branch: the 8 cardinal cubic B-spline basis functions B(y-j)
    (uniform knots) are approximated by 8 Gaussians exp(-k(y-mu_c)^2),
    mu_c = 2..9, k=1.3, fitted by density-weighted least squares on the
    host (rms residual ~2e-3 of basis scale).  The 8x8 recombination A is
    folded into the spline weights, so the spline branch is a dense
    matmul over K = 8*1024 Gaussian features.
  - Each Gaussian feature is computed in ONE ACT op via Derivative_Erf:
    DErf(s*x + b) = (2/sqrt(pi)) * exp(-(s*x+b)^2), written as fp8
    directly.  No DVE subtract/square chain at all.
  - Base branch: Silu ACT table directly (exact silu), fp16 matmul.
  - The spline matmul runs in fp8 (e4m3) DoubleRow (2 k-groups/pass) at
    FD=512.  Weights scaled by S_W to sit in fp8 range; descale in the
    psum drain.
  - ACT table sets: Derivative_Erf and Silu live in different sets; a
    tiny DVE-produced bias tile makes the 8 silu ops depend on the last
    derf op, so ACT order is [64 derf][8 silu] per rep = 2 table loads.
  - Data-parallel over batch: 8 cores x 1024 rows.
"""
import numpy as np
import ml_dtypes

P = 128
NCORES = 8
BATCH, IN_F, OUT_F = 8192, 1024, 1024
B_LOC = BATCH // NCORES          # 1024
N_IC = IN_F // P                 # 8 input-feature chunks
N_OC = OUT_F // P                # 8 output chunks
NG = 8                           # gaussian centers
NCP = NG // 2                    # DoubleRow center pairs
K_G = 1.3                        # gaussian width (y units)

# grid constants (uniform knots; matches reference setup)
GRID_SIZE, SPLINE_ORDER = 5, 3
GRID_LO, GRID_HI = -1.0, 1.0
H = (GRID_HI - GRID_LO) / GRID_SIZE                      # 0.4
T0 = GRID_LO - SPLINE_ORDER * H                          # -2.2
MU_Y = np.arange(2.0, 2.0 + NG)                          # y-space centers
X_MU = (T0 + H * MU_Y).astype(np.float64)                # x-space centers
KP = K_G / (H * H)                                       # x-space width
SC = float(np.sqrt(KP))                                  # derf input scale

_BUILT = {}
_SW = 2048.0   # weight scale; host-verified to keep |w*S_W| < 240


def _fit_A():
    """Fit 8 derf-gaussians to the 8 cardinal basis fns, density weighted."""
    y = np.linspace(-2.0, 13.0, 6001)
    w = np.exp(-0.5 * (H * y + T0) ** 2)        # x-density at y
    sw = np.sqrt(w)
    t = y[:, None] - np.arange(8)[None, :]
    v = 2.0 - np.abs(t - 2.0)
    r1 = np.maximum(v, 0.0); r2 = np.maximum(v - 1.0, 0.0)
    T = (r1 ** 3 - 4.0 * r2 ** 3) / 6.0          # [N, 8] targets
    G = (2.0 / np.sqrt(np.pi)) * np.exp(
        -K_G * (y[:, None] - MU_Y[None, :]) ** 2)          # [N, 8] derf feats
    A, *_ = np.linalg.lstsq(G * sw[:, None], T * sw[:, None], rcond=None)
    return A                                      # [centers, basis]


def _build_nc(repeat=1, unroll=8):
    import concourse.bacc as bacc
    import concourse.mybir as mybir
    from concourse import tile

    AF = mybir.ActivationFunctionType
    ALU = mybir.AluOpType
    F32 = mybir.dt.float32
    F16 = mybir.dt.float16
    FP8 = mybir.dt.float8e4

    descale = float(1.0 / _SW)

    while repeat % unroll:
        unroll //= 2
    unroll = max(unroll, 1)

    nc = bacc.Bacc("TRN2", target_bir_lowering=False, debug=False)

    x_d = nc.dram_tensor("x16", [N_IC, P, B_LOC], F16, kind="ExternalInput")
    wb_d = nc.dram_tensor("wb", [N_OC, P, N_IC, P], F16, kind="ExternalInput")
    wg_d = nc.dram_tensor("wg", [N_OC, P, N_IC * NCP, 2, P], FP8,
                          kind="ExternalInput")
    out_d = nc.dram_tensor("out", [N_OC, P, B_LOC], F32, kind="ExternalOutput")

    with tile.TileContext(nc) as tc:
        with (
            tc.tile_pool(name="consts", bufs=1) as cpool,
            tc.tile_pool(name="xr", bufs=10) as xr,
            tc.tile_pool(name="silp", bufs=2) as silp,
            tc.tile_pool(name="zbp", bufs=2) as zbp,
            tc.tile_pool(name="gp", bufs=2) as gp,
            tc.tile_pool(name="wbp", bufs=1) as wbp,
            tc.tile_pool(name="wgp", bufs=2) as wgp,
            tc.tile_pool(name="op", bufs=2) as op,
            tc.tile_pool(name="psum", bufs=4, space="PSUM") as pp,
        ):
            # per-center derf bias consts
            bc = []
            for c in range(NG):
                b = cpool.tile([P, 1], F32, name=f"bc{c}")
                nc.any.memset(b[:], float(-SC * X_MU[c]))
                bc.append(b)

            # define psum buffers before the loop so iteration-0 "drains of
            # the previous rep" read initialized memory
            for i in range(4):
                t = pp.tile([P, 1024], F32, name=f"pginit{i}", tag="pg")
                nc.vector.memset(t[:], 0.0)

            def emit_drain(prev_pg, oc, rep):
                ot = op.tile([P, 1024], F32, name=f"o{oc}_{rep}", tag="o")
                nc.vector.tensor_scalar(ot[:], prev_pg[oc][:], descale,
                                        None, ALU.mult)
                nc.sync.dma_start(out_d[oc], ot[:])

            def emit_feat(rep):
                """Feature block: x DMA + derf gaussians + silu."""
                xts, gt = [], []
                for ic in range(N_IC):
                    xt = xr.tile([P, B_LOC], F16, name=f"x{ic}_{rep}", tag="x")
                    nc.gpsimd.dma_start(xt[:], x_d[ic])
                    xts.append(xt)
                for ic in range(N_IC):
                    xt = xts[ic]
                    g = gp.tile([P, NG, B_LOC], FP8, name=f"g{ic}_{rep}",
                                tag=f"g{ic}")
                    for c in range(NG):
                        nc.scalar.activation(g[:, c, :], xt[:],
                                             AF.Derivative_Erf,
                                             bias=bc[c][:], scale=SC)
                    gt.append(g)

                # force [derf x64][silu x8] ACT order: silu bias depends on
                # the last derf output (value is exactly 0)
                zb = zbp.tile([P, 1], F32, name=f"zb_{rep}", tag="zb")
                nc.vector.tensor_scalar(zb[:], gt[-1][:, NG - 1, 0:1], 0.0,
                                        None, ALU.mult)
                sil = []
                for ic in range(N_IC):
                    st = silp.tile([P, B_LOC], F16, name=f"sil{ic}_{rep}",
                                   tag=f"s{ic}")
                    nc.scalar.activation(st[:], xts[ic][:], AF.Silu,
                                         bias=zb[:], scale=1.0)
                    sil.append(st)
                return gt, sil

            def emit_mms(rep, feat, prev_pg):
                """Matmul block consuming a feature set; prev_pg: psum tiles
                of the previous rep (oc4..7 still undrained) or None."""
                gt, sil = feat
                if prev_pg is not None:
                    for oc in range(4, 8):
                        emit_drain(prev_pg, oc, rep)

                def mm_oc(oc):
                    wb = wbp.tile([P, N_IC, P], F16, name=f"wb{oc}_{rep}",
                                  tag="wb")
                    nc.sync.dma_start(wb[:], wb_d[oc])
                    wg = wgp.tile([P, N_IC * NCP, 2, P], FP8,
                                  name=f"wg{oc}_{rep}", tag="wg")
                    nc.sync.dma_start(wg[:], wg_d[oc])
                    pg = pp.tile([P, 1024], F32, name=f"pg{oc}_{rep}",
                                 tag="pg")
                    # spline MMs first: they only need the derf block
                    for icp in range(N_IC * NCP):
                        ic, cp = divmod(icp, NCP)
                        for q in range(2):
                            nc.tensor.matmul(
                                pg[:, q * 512:(q + 1) * 512],
                                wg[:, icp, :, :],
                                gt[ic][:, 2 * cp:2 * cp + 2,
                                       q * 512:(q + 1) * 512],
                                start=(icp == 0), stop=False,
                                perf_mode=mybir.MatmulPerfMode.DoubleRow,
                                skip_group_check=True)
                    for ic in range(N_IC):
                        for bh in range(2):
                            nc.tensor.matmul(
                                pg[:, bh * 512:(bh + 1) * 512], wb[:, ic, :],
                                sil[ic][:, bh * 512:(bh + 1) * 512],
                                start=False, stop=(ic == N_IC - 1),
                                skip_group_check=True)
                    return pg

                pgs = {}
                for oc in range(4):
                    pgs[oc] = mm_oc(oc)
                for oc in range(4):
                    emit_drain(pgs, oc, rep)
                for oc in range(4, 8):
                    pgs[oc] = mm_oc(oc)
                return pgs

            def emit_window():
                prev = None
                for w in range(unroll):
                    f = emit_feat(w)
                    prev = emit_mms(w, f, prev)
                for oc in range(4, 8):
                    emit_drain(prev, oc, "tail")

            if repeat == 1:
                emit_window()
            else:
                with tc.For_i(0, repeat // unroll, 1):
                    emit_window()

    nc.compile()
    return nc


def _prep(x, grid, base_weight, spline_weight, spline_scaler):
    # x transposed + fp16
    xT = np.ascontiguousarray(np.asarray(x, np.float32).T).astype(np.float16)

    # base weights: lhsT [oc][ic][p(k), m(out)], pre-scaled by S_W to share
    # the spline psum accumulation
    bw = np.asarray(base_weight, np.float64)             # [out, in]
    wb = (_SW * bw.T).reshape(N_IC, P, N_OC, P)
    wb = np.ascontiguousarray(wb.transpose(2, 1, 0, 3)).astype(np.float16)

    # gaussian weights: W~[i, c, o] = sum_j A[c, j] * (spline_w * scaler)
    A = _fit_A()                                         # [c, j]
    swsc = (np.asarray(spline_weight, np.float64)
            * np.asarray(spline_scaler, np.float64)[:, None, :])  # [in, 8, out]
    Wg = np.einsum('cj,ijo->ico', A, swsc)               # [in, c, out]
    Wgs = Wg * _SW                   # |Wgs| maxes well inside e4m3 range (240)
    # layout [oc, ic*NCP + cp, p, g, m]
    Wgs = Wgs.reshape(N_IC, P, NCP, 2, N_OC, P)          # [ic, p, cp, g, oc, m]
    wg = np.ascontiguousarray(Wgs.transpose(4, 1, 0, 2, 3, 5)).reshape(
        N_OC, P, N_IC * NCP, 2, P).astype(ml_dtypes.float8_e4m3)

    return xT, wb, wg


def _run(nc, in_maps):
    from concourse.bass_utils import run_bass_kernel_spmd
    return run_bass_kernel_spmd(nc, in_maps, core_ids=list(range(NCORES)))


def kernel(x, grid, base_weight, spline_weight, spline_scaler, _repeat=1):
    xT, wb, wg = _prep(x, grid, base_weight, spline_weight, spline_scaler)

    if _repeat not in _BUILT:
        _BUILT[_repeat] = _build_nc(_repeat)
    nc = _BUILT[_repeat]

    in_maps = []
    for c in range(NCORES):
        xs = np.ascontiguousarray(
            xT[:, c * B_LOC:(c + 1) * B_LOC].reshape(N_IC, P, B_LOC))
        in_maps.append({"x16": xs, "wb": wb, "wg": wg})

    res = _run(nc, in_maps)

    out = np.empty((BATCH, OUT_F), np.float32)
    for c in range(NCORES):
        o = res.results[c]["out"].reshape(OUT_F, B_LOC)   # [out, b_loc]
        out[c * B_LOC:(c + 1) * B_LOC, :] = o.T
    return out


# revision 13
# speedup vs baseline: 1.1240x; 1.0273x over previous
"""KANLinear Trainium2 kernel — Derivative_Erf-feature + fp8 DoubleRow version.

Strategy:
  - Spline branch: the 8 cardinal cubic B-spline basis functions B(y-j)
    (uniform knots) are approximated by 8 Gaussians exp(-k(y-mu_c)^2),
    mu_c = 2..9, k=1.3, fitted by density-weighted least squares on the
    host (rms residual ~2e-3 of basis scale).  The 8x8 recombination A is
    folded into the spline weights, so the spline branch is a dense
    matmul over K = 8*1024 Gaussian features.
  - Each Gaussian feature is computed in ONE ACT op via Derivative_Erf:
    DErf(s*x + b) = (2/sqrt(pi)) * exp(-(s*x+b)^2), written as fp8
    directly.  No DVE subtract/square chain at all.
  - Base branch: Silu ACT table directly (exact silu), fp16 matmul.
  - The spline matmul runs in fp8 (e4m3) DoubleRow (2 k-groups/pass) at
    FD=512.  Weights scaled by S_W to sit in fp8 range; descale in the
    psum drain.
  - ACT table sets: Derivative_Erf and Silu live in different sets; a
    tiny DVE-produced bias tile makes the 8 silu ops depend on the last
    derf op, so ACT order is [64 derf][8 silu] per rep = 2 table loads.
  - Data-parallel over batch: 8 cores x 1024 rows.
"""
import numpy as np
import ml_dtypes

P = 128
NCORES = 8
BATCH, IN_F, OUT_F = 8192, 1024, 1024
B_LOC = BATCH // NCORES          # 1024
N_IC = IN_F // P                 # 8 input-feature chunks
N_OC = OUT_F // P                # 8 output chunks
NG = 8                           # gaussian centers
NCP = NG // 2                    # DoubleRow center pairs
K_G = 1.3                        # gaussian width (y units)

# grid constants (uniform knots; matches reference setup)
GRID_SIZE, SPLINE_ORDER = 5, 3
GRID_LO, GRID_HI = -1.0, 1.0
H = (GRID_HI - GRID_LO) / GRID_SIZE                      # 0.4
T0 = GRID_LO - SPLINE_ORDER * H                          # -2.2
MU_Y = np.arange(2.0, 2.0 + NG)                          # y-space centers
X_MU = (T0 + H * MU_Y).astype(np.float64)                # x-space centers
KP = K_G / (H * H)                                       # x-space width
SC = float(np.sqrt(KP))                                  # derf input scale

_BUILT = {}
_SW = 2048.0   # weight scale; host-verified to keep |w*S_W| < 240


def _fit_A():
    """Fit 8 derf-gaussians to the 8 cardinal basis fns, density weighted."""
    y = np.linspace(-2.0, 13.0, 6001)
    w = np.exp(-0.5 * (H * y + T0) ** 2)        # x-density at y
    sw = np.sqrt(w)
    t = y[:, None] - np.arange(8)[None, :]
    v = 2.0 - np.abs(t - 2.0)
    r1 = np.maximum(v, 0.0); r2 = np.maximum(v - 1.0, 0.0)
    T = (r1 ** 3 - 4.0 * r2 ** 3) / 6.0          # [N, 8] targets
    G = (2.0 / np.sqrt(np.pi)) * np.exp(
        -K_G * (y[:, None] - MU_Y[None, :]) ** 2)          # [N, 8] derf feats
    A, *_ = np.linalg.lstsq(G * sw[:, None], T * sw[:, None], rcond=None)
    return A                                      # [centers, basis]


def _build_nc(repeat=1, unroll=16):
    import concourse.bacc as bacc
    import concourse.mybir as mybir
    from concourse import tile

    AF = mybir.ActivationFunctionType
    ALU = mybir.AluOpType
    F32 = mybir.dt.float32
    F16 = mybir.dt.float16
    FP8 = mybir.dt.float8e4

    descale = float(1.0 / _SW)

    while repeat % unroll:
        unroll //= 2
    unroll = max(unroll, 1)

    nc = bacc.Bacc("TRN2", target_bir_lowering=False, debug=False)

    x_d = nc.dram_tensor("x16", [N_IC, P, B_LOC], F16, kind="ExternalInput")
    wb_d = nc.dram_tensor("wb", [N_OC, P, N_IC, P], F16, kind="ExternalInput")
    wg_d = nc.dram_tensor("wg", [N_OC, P, N_IC * NCP, 2, P], FP8,
                          kind="ExternalInput")
    out_d = nc.dram_tensor("out", [N_OC, P, B_LOC], F32, kind="ExternalOutput")

    with tile.TileContext(nc) as tc:
        with (
            tc.tile_pool(name="consts", bufs=1) as cpool,
            tc.tile_pool(name="xr", bufs=10) as xr,
            tc.tile_pool(name="silp", bufs=2) as silp,
            tc.tile_pool(name="zbp", bufs=2) as zbp,
            tc.tile_pool(name="gp", bufs=2) as gp,
            tc.tile_pool(name="wbp", bufs=1) as wbp,
            tc.tile_pool(name="wgp", bufs=2) as wgp,
            tc.tile_pool(name="op", bufs=2) as op,
            tc.tile_pool(name="psum", bufs=4, space="PSUM") as pp,
        ):
            # per-center derf bias consts
            bc = []
            for c in range(NG):
                b = cpool.tile([P, 1], F32, name=f"bc{c}")
                nc.any.memset(b[:], float(-SC * X_MU[c]))
                bc.append(b)

            # define psum buffers before the loop so iteration-0 "drains of
            # the previous rep" read initialized memory
            for i in range(4):
                t = pp.tile([P, 1024], F32, name=f"pginit{i}", tag="pg")
                nc.vector.memset(t[:], 0.0)

            def emit_drain(prev_pg, oc, rep):
                ot = op.tile([P, 1024], F32, name=f"o{oc}_{rep}", tag="o")
                nc.vector.tensor_scalar(ot[:], prev_pg[oc][:], descale,
                                        None, ALU.mult)
                nc.sync.dma_start(out_d[oc], ot[:])

            def emit_feat(rep):
                """Feature block: x DMA + derf gaussians + silu."""
                xts, gt = [], []
                for ic in range(N_IC):
                    xt = xr.tile([P, B_LOC], F16, name=f"x{ic}_{rep}", tag="x")
                    nc.gpsimd.dma_start(xt[:], x_d[ic])
                    xts.append(xt)
                for ic in range(N_IC):
                    xt = xts[ic]
                    g = gp.tile([P, NG, B_LOC], FP8, name=f"g{ic}_{rep}",
                                tag=f"g{ic}")
                    for c in range(NG):
                        nc.scalar.activation(g[:, c, :], xt[:],
                                             AF.Derivative_Erf,
                                             bias=bc[c][:], scale=SC)
                    gt.append(g)

                # force [derf x64][silu x8] ACT order: silu bias depends on
                # the last derf output (value is exactly 0)
                zb = zbp.tile([P, 1], F32, name=f"zb_{rep}", tag="zb")
                nc.vector.tensor_scalar(zb[:], gt[-1][:, NG - 1, 0:1], 0.0,
                                        None, ALU.mult)
                sil = []
                for ic in range(N_IC):
                    st = silp.tile([P, B_LOC], F16, name=f"sil{ic}_{rep}",
                                   tag=f"s{ic}")
                    nc.scalar.activation(st[:], xts[ic][:], AF.Silu,
                                         bias=zb[:], scale=1.0)
                    sil.append(st)
                return gt, sil

            def emit_mms(rep, feat, prev_pg):
                """Matmul block consuming a feature set; prev_pg: psum tiles
                of the previous rep (oc4..7 still undrained) or None."""
                gt, sil = feat
                if prev_pg is not None:
                    for oc in range(4, 8):
                        emit_drain(prev_pg, oc, rep)

                def mm_oc(oc):
                    wb = wbp.tile([P, N_IC, P], F16, name=f"wb{oc}_{rep}",
                                  tag="wb")
                    nc.sync.dma_start(wb[:], wb_d[oc])
                    wg = wgp.tile([P, N_IC * NCP, 2, P], FP8,
                                  name=f"wg{oc}_{rep}", tag="wg")
                    nc.sync.dma_start(wg[:], wg_d[oc])
                    pg = pp.tile([P, 1024], F32, name=f"pg{oc}_{rep}",
                                 tag="pg")
                    # spline MMs first: they only need the derf block
                    for icp in range(N_IC * NCP):
                        ic, cp = divmod(icp, NCP)
                        for q in range(2):
                            nc.tensor.matmul(
                                pg[:, q * 512:(q + 1) * 512],
                                wg[:, icp, :, :],
                                gt[ic][:, 2 * cp:2 * cp + 2,
                                       q * 512:(q + 1) * 512],
                                start=(icp == 0), stop=False,
                                perf_mode=mybir.MatmulPerfMode.DoubleRow,
                                skip_group_check=True)
                    for ic in range(N_IC):
                        for bh in range(2):
                            nc.tensor.matmul(
                                pg[:, bh * 512:(bh + 1) * 512], wb[:, ic, :],
                                sil[ic][:, bh * 512:(bh + 1) * 512],
                                start=False, stop=(ic == N_IC - 1),
                                skip_group_check=True)
                    return pg

                pgs = {}
                for oc in range(4):
                    pgs[oc] = mm_oc(oc)
                for oc in range(4):
                    emit_drain(pgs, oc, rep)
                for oc in range(4, 8):
                    pgs[oc] = mm_oc(oc)
                return pgs

            def emit_window():
                prev = None
                for w in range(unroll):
                    f = emit_feat(w)
                    prev = emit_mms(w, f, prev)
                for oc in range(4, 8):
                    emit_drain(prev, oc, "tail")

            if repeat == 1:
                emit_window()
            else:
                with tc.For_i(0, repeat // unroll, 1):
                    emit_window()

    nc.compile()
    return nc


def _prep(x, grid, base_weight, spline_weight, spline_scaler):
    # x transposed + fp16
    xT = np.ascontiguousarray(np.asarray(x, np.float32).T).astype(np.float16)

    # base weights: lhsT [oc][ic][p(k), m(out)], pre-scaled by S_W to share
    # the spline psum accumulation
    bw = np.asarray(base_weight, np.float64)             # [out, in]
    wb = (_SW * bw.T).reshape(N_IC, P, N_OC, P)
    wb = np.ascontiguousarray(wb.transpose(2, 1, 0, 3)).astype(np.float16)

    # gaussian weights: W~[i, c, o] = sum_j A[c, j] * (spline_w * scaler)
    A = _fit_A()                                         # [c, j]
    swsc = (np.asarray(spline_weight, np.float64)
            * np.asarray(spline_scaler, np.float64)[:, None, :])  # [in, 8, out]
    Wg = np.einsum('cj,ijo->ico', A, swsc)               # [in, c, out]
    Wgs = Wg * _SW                   # |Wgs| maxes well inside e4m3 range (240)
    # layout [oc, ic*NCP + cp, p, g, m]
    Wgs = Wgs.reshape(N_IC, P, NCP, 2, N_OC, P)          # [ic, p, cp, g, oc, m]
    wg = np.ascontiguousarray(Wgs.transpose(4, 1, 0, 2, 3, 5)).reshape(
        N_OC, P, N_IC * NCP, 2, P).astype(ml_dtypes.float8_e4m3)

    return xT, wb, wg


def _run(nc, in_maps):
    from concourse.bass_utils import run_bass_kernel_spmd
    return run_bass_kernel_spmd(nc, in_maps, core_ids=list(range(NCORES)))


def kernel(x, grid, base_weight, spline_weight, spline_scaler, _repeat=1):
    xT, wb, wg = _prep(x, grid, base_weight, spline_weight, spline_scaler)

    if _repeat not in _BUILT:
        _BUILT[_repeat] = _build_nc(_repeat)
    nc = _BUILT[_repeat]

    in_maps = []
    for c in range(NCORES):
        xs = np.ascontiguousarray(
            xT[:, c * B_LOC:(c + 1) * B_LOC].reshape(N_IC, P, B_LOC))
        in_maps.append({"x16": xs, "wb": wb, "wg": wg})

    res = _run(nc, in_maps)

    out = np.empty((BATCH, OUT_F), np.float32)
    for c in range(NCORES):
        o = res.results[c]["out"].reshape(OUT_F, B_LOC)   # [out, b_loc]
        out[c * B_LOC:(c + 1) * B_LOC, :] = o.T
    return out


# revision 14
# speedup vs baseline: 1.1374x; 1.0119x over previous
"""KANLinear Trainium2 kernel — Derivative_Erf-feature + fp8 DoubleRow version.

Strategy:
  - Spline branch: the 8 cardinal cubic B-spline basis functions B(y-j)
    (uniform knots) are approximated by 8 Gaussians exp(-k(y-mu_c)^2),
    mu_c = 2..9, k=1.3, fitted by density-weighted least squares on the
    host (rms residual ~2e-3 of basis scale).  The 8x8 recombination A is
    folded into the spline weights, so the spline branch is a dense
    matmul over K = 8*1024 Gaussian features.
  - Each Gaussian feature is computed in ONE ACT op via Derivative_Erf:
    DErf(s*x + b) = (2/sqrt(pi)) * exp(-(s*x+b)^2), written as fp8
    directly.  No DVE subtract/square chain at all.
  - Base branch: Silu ACT table directly (exact silu), fp16 matmul.
  - The spline matmul runs in fp8 (e4m3) DoubleRow (2 k-groups/pass) at
    FD=512.  Weights scaled by S_W to sit in fp8 range; descale in the
    psum drain.
  - ACT table sets: Derivative_Erf and Silu live in different sets; a
    tiny DVE-produced bias tile makes the 8 silu ops depend on the last
    derf op, so ACT order is [64 derf][8 silu] per rep = 2 table loads.
  - Data-parallel over batch: 8 cores x 1024 rows.
"""
import numpy as np
import ml_dtypes

P = 128
NCORES = 8
BATCH, IN_F, OUT_F = 8192, 1024, 1024
B_LOC = BATCH // NCORES          # 1024
N_IC = IN_F // P                 # 8 input-feature chunks
N_OC = OUT_F // P                # 8 output chunks
NG = 8                           # gaussian centers
NCP = NG // 2                    # DoubleRow center pairs
K_G = 1.3                        # gaussian width (y units)

# grid constants (uniform knots; matches reference setup)
GRID_SIZE, SPLINE_ORDER = 5, 3
GRID_LO, GRID_HI = -1.0, 1.0
H = (GRID_HI - GRID_LO) / GRID_SIZE                      # 0.4
T0 = GRID_LO - SPLINE_ORDER * H                          # -2.2
MU_Y = np.arange(2.0, 2.0 + NG)                          # y-space centers
X_MU = (T0 + H * MU_Y).astype(np.float64)                # x-space centers
KP = K_G / (H * H)                                       # x-space width
SC = float(np.sqrt(KP))                                  # derf input scale

_BUILT = {}
_SW = 2048.0   # weight scale; host-verified to keep |w*S_W| < 240


def _fit_A():
    """Fit 8 derf-gaussians to the 8 cardinal basis fns, density weighted."""
    y = np.linspace(-2.0, 13.0, 6001)
    w = np.exp(-0.5 * (H * y + T0) ** 2)        # x-density at y
    sw = np.sqrt(w)
    t = y[:, None] - np.arange(8)[None, :]
    v = 2.0 - np.abs(t - 2.0)
    r1 = np.maximum(v, 0.0); r2 = np.maximum(v - 1.0, 0.0)
    T = (r1 ** 3 - 4.0 * r2 ** 3) / 6.0          # [N, 8] targets
    G = (2.0 / np.sqrt(np.pi)) * np.exp(
        -K_G * (y[:, None] - MU_Y[None, :]) ** 2)          # [N, 8] derf feats
    A, *_ = np.linalg.lstsq(G * sw[:, None], T * sw[:, None], rcond=None)
    return A                                      # [centers, basis]


def _build_nc(repeat=1, unroll=32):
    import concourse.bacc as bacc
    import concourse.mybir as mybir
    from concourse import tile

    AF = mybir.ActivationFunctionType
    ALU = mybir.AluOpType
    F32 = mybir.dt.float32
    F16 = mybir.dt.float16
    FP8 = mybir.dt.float8e4

    descale = float(1.0 / _SW)

    while repeat % unroll:
        unroll //= 2
    unroll = max(unroll, 1)

    nc = bacc.Bacc("TRN2", target_bir_lowering=False, debug=False)

    x_d = nc.dram_tensor("x16", [N_IC, P, B_LOC], F16, kind="ExternalInput")
    wb_d = nc.dram_tensor("wb", [N_OC, P, N_IC, P], F16, kind="ExternalInput")
    wg_d = nc.dram_tensor("wg", [N_OC, P, N_IC * NCP, 2, P], FP8,
                          kind="ExternalInput")
    out_d = nc.dram_tensor("out", [N_OC, P, B_LOC], F32, kind="ExternalOutput")

    with tile.TileContext(nc) as tc:
        with (
            tc.tile_pool(name="consts", bufs=1) as cpool,
            tc.tile_pool(name="xr", bufs=10) as xr,
            tc.tile_pool(name="silp", bufs=2) as silp,
            tc.tile_pool(name="zbp", bufs=2) as zbp,
            tc.tile_pool(name="gp", bufs=2) as gp,
            tc.tile_pool(name="wbp", bufs=1) as wbp,
            tc.tile_pool(name="wgp", bufs=2) as wgp,
            tc.tile_pool(name="op", bufs=2) as op,
            tc.tile_pool(name="psum", bufs=4, space="PSUM") as pp,
        ):
            # per-center derf bias consts
            bc = []
            for c in range(NG):
                b = cpool.tile([P, 1], F32, name=f"bc{c}")
                nc.any.memset(b[:], float(-SC * X_MU[c]))
                bc.append(b)

            # define psum buffers before the loop so iteration-0 "drains of
            # the previous rep" read initialized memory
            for i in range(4):
                t = pp.tile([P, 1024], F32, name=f"pginit{i}", tag="pg")
                nc.vector.memset(t[:], 0.0)

            def emit_drain(prev_pg, oc, rep):
                ot = op.tile([P, 1024], F32, name=f"o{oc}_{rep}", tag="o")
                nc.vector.tensor_scalar(ot[:], prev_pg[oc][:], descale,
                                        None, ALU.mult)
                nc.sync.dma_start(out_d[oc], ot[:])

            def emit_feat(rep):
                """Feature block: x DMA + derf gaussians + silu."""
                xts, gt = [], []
                for ic in range(N_IC):
                    xt = xr.tile([P, B_LOC], F16, name=f"x{ic}_{rep}", tag="x")
                    nc.gpsimd.dma_start(xt[:], x_d[ic])
                    xts.append(xt)
                for ic in range(N_IC):
                    xt = xts[ic]
                    g = gp.tile([P, NG, B_LOC], FP8, name=f"g{ic}_{rep}",
                                tag=f"g{ic}")
                    for c in range(NG):
                        nc.scalar.activation(g[:, c, :], xt[:],
                                             AF.Derivative_Erf,
                                             bias=bc[c][:], scale=SC)
                    gt.append(g)

                # force [derf x64][silu x8] ACT order: silu bias depends on
                # the last derf output (value is exactly 0)
                zb = zbp.tile([P, 1], F32, name=f"zb_{rep}", tag="zb")
                nc.vector.tensor_scalar(zb[:], gt[-1][:, NG - 1, 0:1], 0.0,
                                        None, ALU.mult)
                sil = []
                for ic in range(N_IC):
                    st = silp.tile([P, B_LOC], F16, name=f"sil{ic}_{rep}",
                                   tag=f"s{ic}")
                    nc.scalar.activation(st[:], xts[ic][:], AF.Silu,
                                         bias=zb[:], scale=1.0)
                    sil.append(st)
                return gt, sil

            def emit_mms(rep, feat, prev_pg):
                """Matmul block consuming a feature set; prev_pg: psum tiles
                of the previous rep (oc4..7 still undrained) or None."""
                gt, sil = feat
                if prev_pg is not None:
                    for oc in range(4, 8):
                        emit_drain(prev_pg, oc, rep)

                def mm_oc(oc):
                    wb = wbp.tile([P, N_IC, P], F16, name=f"wb{oc}_{rep}",
                                  tag="wb")
                    nc.sync.dma_start(wb[:], wb_d[oc])
                    wg = wgp.tile([P, N_IC * NCP, 2, P], FP8,
                                  name=f"wg{oc}_{rep}", tag="wg")
                    nc.sync.dma_start(wg[:], wg_d[oc])
                    pg = pp.tile([P, 1024], F32, name=f"pg{oc}_{rep}",
                                 tag="pg")
                    # spline MMs first: they only need the derf block
                    for icp in range(N_IC * NCP):
                        ic, cp = divmod(icp, NCP)
                        for q in range(2):
                            nc.tensor.matmul(
                                pg[:, q * 512:(q + 1) * 512],
                                wg[:, icp, :, :],
                                gt[ic][:, 2 * cp:2 * cp + 2,
                                       q * 512:(q + 1) * 512],
                                start=(icp == 0), stop=False,
                                perf_mode=mybir.MatmulPerfMode.DoubleRow,
                                skip_group_check=True)
                    for ic in range(N_IC):
                        for bh in range(2):
                            nc.tensor.matmul(
                                pg[:, bh * 512:(bh + 1) * 512], wb[:, ic, :],
                                sil[ic][:, bh * 512:(bh + 1) * 512],
                                start=False, stop=(ic == N_IC - 1),
                                skip_group_check=True)
                    return pg

                pgs = {}
                for oc in range(4):
                    pgs[oc] = mm_oc(oc)
                for oc in range(4):
                    emit_drain(pgs, oc, rep)
                for oc in range(4, 8):
                    pgs[oc] = mm_oc(oc)
                return pgs

            def emit_window():
                prev = None
                for w in range(unroll):
                    f = emit_feat(w)
                    prev = emit_mms(w, f, prev)
                for oc in range(4, 8):
                    emit_drain(prev, oc, "tail")

            if repeat == 1:
                emit_window()
            else:
                with tc.For_i(0, repeat // unroll, 1):
                    emit_window()

    nc.compile()
    return nc


def _prep(x, grid, base_weight, spline_weight, spline_scaler):
    # x transposed + fp16
    xT = np.ascontiguousarray(np.asarray(x, np.float32).T).astype(np.float16)

    # base weights: lhsT [oc][ic][p(k), m(out)], pre-scaled by S_W to share
    # the spline psum accumulation
    bw = np.asarray(base_weight, np.float64)             # [out, in]
    wb = (_SW * bw.T).reshape(N_IC, P, N_OC, P)
    wb = np.ascontiguousarray(wb.transpose(2, 1, 0, 3)).astype(np.float16)

    # gaussian weights: W~[i, c, o] = sum_j A[c, j] * (spline_w * scaler)
    A = _fit_A()                                         # [c, j]
    swsc = (np.asarray(spline_weight, np.float64)
            * np.asarray(spline_scaler, np.float64)[:, None, :])  # [in, 8, out]
    Wg = np.einsum('cj,ijo->ico', A, swsc)               # [in, c, out]
    Wgs = Wg * _SW                   # |Wgs| maxes well inside e4m3 range (240)
    # layout [oc, ic*NCP + cp, p, g, m]
    Wgs = Wgs.reshape(N_IC, P, NCP, 2, N_OC, P)          # [ic, p, cp, g, oc, m]
    wg = np.ascontiguousarray(Wgs.transpose(4, 1, 0, 2, 3, 5)).reshape(
        N_OC, P, N_IC * NCP, 2, P).astype(ml_dtypes.float8_e4m3)

    return xT, wb, wg


def _run(nc, in_maps):
    from concourse.bass_utils import run_bass_kernel_spmd
    return run_bass_kernel_spmd(nc, in_maps, core_ids=list(range(NCORES)))


def kernel(x, grid, base_weight, spline_weight, spline_scaler, _repeat=1):
    xT, wb, wg = _prep(x, grid, base_weight, spline_weight, spline_scaler)

    if _repeat not in _BUILT:
        _BUILT[_repeat] = _build_nc(_repeat)
    nc = _BUILT[_repeat]

    in_maps = []
    for c in range(NCORES):
        xs = np.ascontiguousarray(
            xT[:, c * B_LOC:(c + 1) * B_LOC].reshape(N_IC, P, B_LOC))
        in_maps.append({"x16": xs, "wb": wb, "wg": wg})

    res = _run(nc, in_maps)

    out = np.empty((BATCH, OUT_F), np.float32)
    for c in range(NCORES):
        o = res.results[c]["out"].reshape(OUT_F, B_LOC)   # [out, b_loc]
        out[c * B_LOC:(c + 1) * B_LOC, :] = o.T
    return out
